# revision 33
# baseline (speedup 1.0000x reference)
"""Trainium2 Bass kernel for nn_Attention_3410204033597.

AlphaFold-style gated attention with pair bias:
  B=2, L=1024, E=1024, H=16 heads, head_dim=64
  qkv proj -> per-head scores (q*scale) @ k^T + bias -> softmax
  -> @ v -> gate sigmoid(x@Wg^T + bg) -> o-proj -> (y, attn)

Sharding: 8 cores, each handles one (batch, lq-slice-of-256) block:
  core c: batch b=c//4, query rows [s*256, (s+1)*256), s=c%4.
  Each core recomputes full k/v for its batch (no cross-core comms),
  computes all 16 heads for its query rows, and produces complete
  output rows for both `y` and `attn` -> host assembly is pure
  concatenation.

All heavy matmuls run in float32r (full-speed PE mode for N>=256,
~1.5e-4 rel error vs fp32; raw fp32 bytes are accepted when the DRAM
tensor/tile dtype is declared float32r, and ACT/DVE ops writing into
float32r tiles act as the rounding producers walrus requires).
Softmax: bias-add on DVE (pair bias is pre-transposed to (H, Lq, Lk)
on the host so tiles load contiguously), exp + row-sum via one ACT
activation with accum_out, reciprocal + normalize on DVE, transpose on
PE in f32r (needed anyway because the `attn` output is the transposed
softmax matrix), and the transposed tiles feed the AV matmul directly
as the moving operand (lhsT = v in natural (lk, c) layout). End-to-end
rel err vs the fp32 reference: ~3.5e-4; ~300 us per invocation on 8
cores (wall-clock-slope measured).
"""

import numpy as np
from contextlib import ExitStack

import concourse.bass as bass
import concourse.tile as tile
from concourse import bacc, mybir
from concourse.bass_utils import run_bass_kernel_spmd
from concourse.masks import make_identity

B, L, E, H, HW = 2, 1024, 1024, 16, 64
P = 128
NCORES = 8
SLICES = 4          # lq slices per batch
LS = L // SLICES    # 256 query rows per core
EO = E // P         # 8 chunks of the embed dim
LKO = L // P        # 8 chunks of the key dim
F32 = mybir.dt.float32
F32R = mybir.dt.float32r
BF16 = mybir.dt.bfloat16
AF = mybir.ActivationFunctionType
ALU = mybir.AluOpType
WDT = F32R  # dtype for weights + x (projection matmul operands)

_NC_CACHE = None


def _build(repeat=None, internal_io=False, stages=("p1", "sc", "sm", "tr", "av", "p3"),
           fused_bias=True, gp_bias=False, deep_bufs=False,
           wide=False, act_norm=False, ldw_amortize=False):
    """Build the SPMD program. internal_io=True replaces the big I/O tensors
    with device-resident Internal DRAM (garbage contents) plus a dummy
    1-element output -- used only for wall-clock slope benchmarking where
    host<->device transfer must be excluded. repeat=N wraps the whole body
    in a For_i loop for slope timing. stages: subset for perf bisection
    (p1=projections, sc=scores+bias, sm=exp/norm, tr=transpose+attn-out,
    av=AV, p3=gating+o-proj)."""
    stages = set(stages)
    nc = bacc.Bacc("TRN2", target_bir_lowering=False, debug=False,
                   num_devices=NCORES)

    if internal_io:
        def din(name, shape, dt):
            return nc.dram_tensor(name, shape, dt).ap()
        def dout(name, shape, dt):
            return nc.dram_tensor(name, shape, dt).ap()
    else:
        def din(name, shape, dt):
            return nc.dram_tensor(name, shape, dt, kind="ExternalInput").ap()
        def dout(name, shape, dt):
            return nc.dram_tensor(name, shape, dt, kind="ExternalOutput").ap()

    xT = din("xT", (E, L), WDT)         # x[b].T
    xslT = din("xslT", (E, LS), WDT)    # x[b, lq_slice].T
    wqT = din("wqT", (E, E), WDT)       # scale * Wq^T, cols ordered (h, c)
    wkT = din("wkT", (E, E), WDT)
    wvT = din("wvT", (E, E), WDT)
    wgT = din("wgT", (E, E), WDT)
    woT = din("woT", (E, E), WDT)
    bgp = din("bgp", (P, EO), F32)      # bg[fo*128+p] at [p, fo]
    bob = din("bob", (P, E), F32)       # bo broadcast over partitions
    # declared f32r so it can be accumulated into the scores PSUM via an
    # identity matmul (bytes are plain fp32 from the host)
    biasT = din("biasT", (H, LS, L), F32R)  # bias[b,:,:,h] sliced+transposed
    attnT = dout("attnT", (L, H, LS), F32)
    yOut = dout("yOut", (LS, E), F32)
    if internal_io:
        marker = nc.dram_tensor("marker", (1, 1), F32,
                                kind="ExternalOutput").ap()

    xT_r = xT.rearrange("(eo p) l -> p eo l", p=P)
    xslT_r = xslT.rearrange("(eo p) l -> p eo l", p=P)
    wq_r = wqT.rearrange("(eo p) f -> p eo f", p=P)
    wk_r = wkT.rearrange("(eo p) f -> p eo f", p=P)
    wv_r = wvT.rearrange("(eo p) f -> p eo f", p=P)
    wg_r = wgT.rearrange("(eo p) f -> p eo f", p=P)
    wo_r = woT.rearrange("(eo p) f -> p eo f", p=P)
    biasT_r = biasT.rearrange("h (lqc p) lk -> h p lqc lk", p=P)
    attnT_r = attnT.rearrange("(lko p) h lq -> p lko h lq", p=P)
    yOut_r = yOut.rearrange("(lqc p) e -> p lqc e", p=P)

    with tile.TileContext(nc) as tc, ExitStack() as ctx:
        if repeat is not None:
            ctx.enter_context(tc.For_i(0, repeat))
        const = ctx.enter_context(tc.tile_pool(name="const", bufs=1))
        persist = ctx.enter_context(tc.tile_pool(name="persist", bufs=1))

        ident = const.tile([P, P], F32)
        make_identity(nc, ident[:])
        # f32r copy of the identity (0/1 are exact): lhsT for the PE
        # bias-accumulate matmul and for f32r transposes
        identR = const.tile([P, P], F32R)
        nc.vector.tensor_copy(identR[:], ident[:])
        bgp_sb = const.tile([P, EO], F32)
        nc.sync.dma_start(bgp_sb[:], bgp)
        bob_sb = const.tile([P, E], F32)
        nc.sync.dma_start(bob_sb[:], bob)

        # persistent intermediates
        kT_sb = persist.tile([P, EO, L], F32R)    # (f%128, fo, lk)
        v_sb = persist.tile([P, LKO, E], F32R)    # (lk%128, lko, f)
        qT_sb = persist.tile([P, EO, LS], F32R)   # (f%128, fo, lq)
        gT_sb = persist.tile([P, EO, LS], F32)
        yT_sb = persist.tile([P, EO, LS], F32)

        # ---------------- phase 1: projections ----------------
        with tc.tile_pool(name="xpool", bufs=1) as xpool, \
             tc.tile_pool(name="wpool", bufs=2) as wpool, \
             tc.tile_pool(name="ps1", bufs=4, space="PSUM") as ps1:
          if "p1" in stages:
            xT_sb = xpool.tile([P, EO, L], WDT)
            nc.sync.dma_start(xT_sb[:], xT_r)
            xslT_sb = xpool.tile([P, EO, LS], WDT)
            nc.sync.dma_start(xslT_sb[:], xslT_r)

            # k^T: out (f-chunk, lk)
            wk_sb = wpool.tile([P, EO, E], WDT, tag="w")
            nc.sync.dma_start(wk_sb[:], wk_r)
            for fo in range(EO):
                if ldw_amortize:
                    psts = [ps1.tile([P, 512], F32, tag="pst", name=f"kp{nh}") for nh in range(2)]
                    for eo in range(EO):
                        for nh in range(2):
                            nc.tensor.matmul(
                                psts[nh][:], wk_sb[:, eo, fo * P:(fo + 1) * P],
                                xT_sb[:, eo, nh * 512:(nh + 1) * 512],
                                start=(eo == 0), stop=(eo == EO - 1))
                    for nh in range(2):
                        if (fo + nh) % 2 == 0:
                            nc.scalar.copy(kT_sb[:, fo, nh * 512:(nh + 1) * 512], psts[nh][:])
                        else:
                            nc.vector.tensor_copy(kT_sb[:, fo, nh * 512:(nh + 1) * 512], psts[nh][:])
                else:
                    for nh in range(2):
                        pst = ps1.tile([P, 512], F32, tag="pst")
                        for eo in range(EO):
                            nc.tensor.matmul(
                                pst[:], wk_sb[:, eo, fo * P:(fo + 1) * P],
                                xT_sb[:, eo, nh * 512:(nh + 1) * 512],
                                start=(eo == 0), stop=(eo == EO - 1))
                        if (fo + nh) % 2 == 0:
                            nc.scalar.copy(kT_sb[:, fo, nh * 512:(nh + 1) * 512], pst[:])
                        else:
                            nc.vector.tensor_copy(kT_sb[:, fo, nh * 512:(nh + 1) * 512], pst[:])

            # v: out (lk-chunk, f)
            wv_sb = wpool.tile([P, EO, E], WDT, tag="w")
            nc.sync.dma_start(wv_sb[:], wv_r)
            for lko in range(LKO):
                if ldw_amortize:
                    psts = [ps1.tile([P, 512], F32, tag="pst", name=f"vp{nh}") for nh in range(2)]
                    for eo in range(EO):
                        for nh in range(2):
                            nc.tensor.matmul(
                                psts[nh][:], xT_sb[:, eo, lko * P:(lko + 1) * P],
                                wv_sb[:, eo, nh * 512:(nh + 1) * 512],
                                start=(eo == 0), stop=(eo == EO - 1))
                    for nh in range(2):
                        if (lko + nh) % 2 == 0:
                            nc.scalar.copy(v_sb[:, lko, nh * 512:(nh + 1) * 512], psts[nh][:])
                        else:
                            nc.vector.tensor_copy(v_sb[:, lko, nh * 512:(nh + 1) * 512], psts[nh][:])
                else:
                    for nh in range(2):
                        pst = ps1.tile([P, 512], F32, tag="pst")
                        for eo in range(EO):
                            nc.tensor.matmul(
                                pst[:], xT_sb[:, eo, lko * P:(lko + 1) * P],
                                wv_sb[:, eo, nh * 512:(nh + 1) * 512],
                                start=(eo == 0), stop=(eo == EO - 1))
                        if (lko + nh) % 2 == 0:
                            nc.scalar.copy(v_sb[:, lko, nh * 512:(nh + 1) * 512], pst[:])
                        else:
                            nc.vector.tensor_copy(v_sb[:, lko, nh * 512:(nh + 1) * 512], pst[:])

            # q^T: out (f-chunk, lq_slice)
            wq_sb = wpool.tile([P, EO, E], WDT, tag="w")
            nc.sync.dma_start(wq_sb[:], wq_r)
            for fo in range(EO):
                pst = ps1.tile([P, 512], F32, tag="pst")
                for eo in range(EO):
                    nc.tensor.matmul(
                        pst[:, :LS], wq_sb[:, eo, fo * P:(fo + 1) * P],
                        xslT_sb[:, eo, :],
                        start=(eo == 0), stop=(eo == EO - 1))
                if fo % 2 == 0:
                    nc.scalar.copy(qT_sb[:, fo, :], pst[:, :LS])
                else:
                    nc.vector.tensor_copy(qT_sb[:, fo, :], pst[:, :LS])

            # g^T with fused sigmoid(x@Wg^T + bg)
            wg_sb = wpool.tile([P, EO, E], WDT, tag="w")
            nc.sync.dma_start(wg_sb[:], wg_r)
            for fo in range(EO):
                pst = ps1.tile([P, 512], F32, tag="pst")
                for eo in range(EO):
                    nc.tensor.matmul(
                        pst[:, :LS], wg_sb[:, eo, fo * P:(fo + 1) * P],
                        xslT_sb[:, eo, :],
                        start=(eo == 0), stop=(eo == EO - 1))
                nc.scalar.activation(gT_sb[:, fo, :], pst[:, :LS], AF.Sigmoid,
                                     bias=bgp_sb[:, fo:fo + 1])

        # ---------------- phase 2: per-head attention ----------------
        _smb = 4 if deep_bufs else 3
        _pss = 2 if fused_bias else 4
        _wb = 4 if wide else 3
        with tc.tile_pool(name="bias_p", bufs=_wb) as bias_p, \
             tc.tile_pool(name="sm", bufs=_smb) as sm, \
             tc.tile_pool(name="aT_p", bufs=_wb) as aT_p, \
             tc.tile_pool(name="zp", bufs=4) as zp, \
             tc.tile_pool(name="ps_s", bufs=_pss, space="PSUM") as ps_s, \
             tc.tile_pool(name="ps_t", bufs=2, space="PSUM") as ps_t, \
             tc.tile_pool(name="ps_y", bufs=2, space="PSUM") as ps_y:
            for h in range(H):
                hp, ho = (h % 2) * HW, h // 2
                bias_h = bias_p.tile([P, 2, L], F32R)
                if "sc" in stages:
                    nc.sync.dma_start(bias_h[:], biasT_r[h])
                aT_sb = aT_p.tile([P, LKO, LS], F32R)
                for lqc in range(2):
                    s_sb = sm.tile([P, L], F32, tag="s")
                    if "sc" in stages and fused_bias:
                        pst = ps_s.tile([P, L], F32)
                        for lkh in range(2):
                            nc.tensor.matmul(
                                pst[:, lkh * 512:(lkh + 1) * 512],
                                qT_sb[hp:hp + HW, ho, lqc * P:(lqc + 1) * P],
                                kT_sb[hp:hp + HW, ho, lkh * 512:(lkh + 1) * 512],
                                start=True, stop=True)
                        beng = nc.gpsimd if (gp_bias and h % 2 == 1) else nc.vector
                        beng.tensor_tensor(
                            s_sb[:], pst[:],
                            bias_h[:, lqc, :].bitcast(F32), ALU.add)
                    elif "sc" in stages:
                        for lkh in range(2):
                            pst = ps_s.tile([P, 512], F32)
                            nc.tensor.matmul(
                                pst[:],
                                qT_sb[hp:hp + HW, ho, lqc * P:(lqc + 1) * P],
                                kT_sb[hp:hp + HW, ho, lkh * 512:(lkh + 1) * 512],
                                start=True, stop=True)
                            beng = nc.gpsimd if (gp_bias and h % 2 == 1) else nc.vector
                            beng.tensor_tensor(
                                s_sb[:, lkh * 512:(lkh + 1) * 512], pst[:],
                                bias_h[:, lqc, lkh * 512:(lkh + 1) * 512].bitcast(F32),
                                ALU.add)
                    e_sb = sm.tile([P, L], F32R, tag="e")
                    if "sm" in stages:
                        zc = zp.tile([P, 1], F32, tag="z")
                        rz = zp.tile([P, 1], F32, tag="rz")
                        nc.scalar.activation(e_sb[:], s_sb[:], AF.Exp,
                                             accum_out=zc[:])
                        nc.vector.reciprocal(rz[:], zc[:])
                        if act_norm and h % 2 == 1:
                            nc.scalar.mul(e_sb[:], e_sb[:], rz[:, 0:1])
                        else:
                            nc.vector.tensor_scalar_mul(e_sb[:], e_sb[:], rz[:])
                    if "tr" in stages:
                        # transpose in pairs -> one (128, 256) psum tile,
                        # one copyback per pair, alternating DVE/ACT
                        for lkp in range(LKO // 2):
                            tp = ps_t.tile([P, 2 * P], F32R)
                            for j in range(2):
                                nc.tensor.transpose(
                                    tp[:, j * P:(j + 1) * P],
                                    e_sb[:, (2 * lkp + j) * P:(2 * lkp + j + 1) * P],
                                    identR[:])
                            dst = aT_sb[:, 2 * lkp:2 * lkp + 2,
                                        lqc * P:(lqc + 1) * P]
                            src = tp[:].rearrange("p (k l) -> p k l", k=2)
                            if lkp % 2 == 0:
                                nc.vector.tensor_copy(dst, src)
                            else:
                                nc.scalar.copy(dst, src)
                # AV: y^T_h (c=64, lq)
                if "av" in stages:
                    yp = ps_y.tile([HW, LS], F32)
                    for lko in range(LKO):
                        nc.tensor.matmul(
                            yp[:], v_sb[:, lko, h * HW:(h + 1) * HW],
                            aT_sb[:, lko, :],
                            start=(lko == 0), stop=(lko == LKO - 1))
                    nc.scalar.copy(yT_sb[hp:hp + HW, ho, :], yp[:])
                if "tr" in stages:
                    nc.gpsimd.dma_start(attnT_r[:, :, h, :], aT_sb[:].bitcast(F32))

        # ---------------- phase 3: gating + o-proj ----------------
        with tc.tile_pool(name="ph3b", bufs=2) as ph3b, \
             tc.tile_pool(name="ps3", bufs=4, space="PSUM") as ps3:
          if "p3" in stages:
            wo_sb = ph3b.tile([P, EO, E], WDT, tag="w3")
            nc.sync.dma_start(wo_sb[:], wo_r)
            ygT = ph3b.tile([P, EO, LS], WDT, tag="yg")
            nc.vector.tensor_tensor(ygT[:], yT_sb[:], gT_sb[:], ALU.mult)
            for lqc in range(2):
                for nh in range(2):
                    pst = ps3.tile([P, 512], F32)
                    for fo in range(EO):
                        nc.tensor.matmul(
                            pst[:], ygT[:, fo, lqc * P:(lqc + 1) * P],
                            wo_sb[:, fo, nh * 512:(nh + 1) * 512],
                            start=(fo == 0), stop=(fo == EO - 1))
                    o_sb = ph3b.tile([P, 512], F32, tag="o")
                    nc.vector.tensor_tensor(
                        o_sb[:], pst[:], bob_sb[:, nh * 512:(nh + 1) * 512],
                        ALU.add)
                    nc.sync.dma_start(yOut_r[:, lqc, nh * 512:(nh + 1) * 512],
                                      o_sb[:])

        if internal_io:
            one = const.tile([1, 1], F32)
            nc.vector.memset(one[:], 1.0)
            nc.sync.dma_start(marker[:], one[:])

    nc.compile()
    return nc


def _get_nc():
    global _NC_CACHE
    if _NC_CACHE is None:
        _NC_CACHE = _build()
    return _NC_CACHE


def kernel(**inputs):
    x = np.ascontiguousarray(np.asarray(inputs["x"]), dtype=np.float32)
    mask = np.asarray(inputs["mask"])
    bias = np.asarray(inputs["bias"], dtype=np.float32)
    Wqkv = np.asarray(inputs["Wqkv"], dtype=np.float32)
    Wo = np.asarray(inputs["Wo"], dtype=np.float32)
    bo = np.asarray(inputs["bo"], dtype=np.float32)
    Wg = np.asarray(inputs["Wg"], dtype=np.float32)
    bg = np.asarray(inputs["bg"], dtype=np.float32)

    import ml_dtypes
    wnp = ml_dtypes.bfloat16 if WDT == BF16 else np.float32
    scale = HW ** -0.5
    Wr = Wqkv.reshape(H, 3, HW, E)
    wqT = np.ascontiguousarray((Wr[:, 0].reshape(H * HW, E) * scale).T.astype(wnp))
    wkT = np.ascontiguousarray(Wr[:, 1].reshape(H * HW, E).T.astype(wnp))
    wvT = np.ascontiguousarray(Wr[:, 2].reshape(H * HW, E).T.astype(wnp))
    wgT = np.ascontiguousarray(Wg.T.astype(wnp))
    woT = np.ascontiguousarray(Wo.T.astype(wnp))
    bgp = np.ascontiguousarray(bg.reshape(EO, P).T)
    bob = np.ascontiguousarray(np.broadcast_to(bo, (P, E)))

    if not mask.all():
        # masked keys -> -1e30 on their score columns (exp underflows to 0,
        # matching the reference's -inf semantics)
        bias = bias + np.where(mask[:, None, :, None], 0.0, -1e30).astype(
            np.float32)

    in_maps = []
    for c in range(NCORES):
        b, s = divmod(c, SLICES)
        bT = np.ascontiguousarray(
            bias[b].transpose(2, 0, 1)[:, s * LS:(s + 1) * LS, :])
        in_maps.append({
            "xT": np.ascontiguousarray(x[b].T.astype(wnp)),
            "xslT": np.ascontiguousarray(x[b, s * LS:(s + 1) * LS].T.astype(wnp)),
            "wqT": wqT, "wkT": wkT, "wvT": wvT, "wgT": wgT, "woT": woT,
            "bgp": bgp, "bob": bob,
            "biasT": bT,
        })

    nc = _get_nc()
    res = run_bass_kernel_spmd(nc, in_maps, core_ids=list(range(NCORES)))

    y = np.empty((B, L, E), dtype=np.float32)
    attn = np.empty((B, L, H, L), dtype=np.float32)
    for c in range(NCORES):
        b, s = divmod(c, SLICES)
        y[b, s * LS:(s + 1) * LS] = res.results[c]["yOut"]
        attn[b, :, :, s * LS:(s + 1) * LS] = res.results[c]["attnT"]
    return y, attn


# revision 49
# speedup vs baseline: 1.4207x; 1.4207x over previous
"""Trainium2 Bass kernel for nn_Attention_3410204033597.

AlphaFold-style gated attention with pair bias:
  B=2, L=1024, E=1024, H=16 heads, head_dim=64
  qkv proj -> per-head scores (q*scale) @ k^T + bias -> softmax
  -> @ v -> gate sigmoid(x@Wg^T + bg) -> o-proj -> (y, attn)

Sharding: 8 cores, each handles one (batch, lq-slice-of-256) block:
  core c: batch b=c//4, query rows [s*256, (s+1)*256), s=c%4.
  Each core recomputes full k/v for its batch (no cross-core comms),
  computes all 16 heads for its query rows, and produces complete
  output rows for both `y` and `attn` -> host assembly is pure
  concatenation.

All heavy matmuls run in float32r (full-speed PE mode for N>=256,
~1.5e-4 rel error vs fp32; raw fp32 bytes are accepted when the DRAM
tensor/tile dtype is declared float32r, and ACT/DVE ops writing into
float32r tiles act as the rounding producers walrus requires).
Softmax: one fused (128,1024) DVE bias-add per row-block (pair bias is
pre-transposed to (H, Lq, Lk) on the host so tiles load contiguously,
and the scores land in a two-bank PSUM tile), exp + row-sum via one ACT
activation with accum_out, reciprocal + normalize on DVE, transpose on
PE in f32r (needed anyway because the `attn` output is the transposed
softmax matrix), and the transposed tiles feed the AV matmul directly
as the moving operand (lhsT = v in natural (lk, c) layout). End-to-end
rel err vs the fp32 reference: ~3.5e-4; ~213 us per invocation on 8
cores per the cost-model timeline sim (which tracks hardware within
noise; HW wall-slope measurements land in the 220-290 us band with
+-15% device variance). Input DMAs are split per contraction chunk so
projection matmuls start as soon as the first chunks land.
"""

import numpy as np
from contextlib import ExitStack

import concourse.bass as bass
import concourse.tile as tile
from concourse import bacc, mybir
from concourse.bass_utils import run_bass_kernel_spmd
from concourse.masks import make_identity

B, L, E, H, HW = 2, 1024, 1024, 16, 64
P = 128
NCORES = 8
SLICES = 4          # lq slices per batch
LS = L // SLICES    # 256 query rows per core
EO = E // P         # 8 chunks of the embed dim
LKO = L // P        # 8 chunks of the key dim
F32 = mybir.dt.float32
F32R = mybir.dt.float32r
BF16 = mybir.dt.bfloat16
AF = mybir.ActivationFunctionType
ALU = mybir.AluOpType
WDT = F32R  # dtype for weights + x (projection matmul operands)

_NC_CACHE = None


def _build(repeat=None, internal_io=False, stages=("p1", "sc", "sm", "tr", "av", "p3"),
           fused_bias=True, gp_bias=False, deep_bufs=False,
           wide=False, act_norm=False, ldw_amortize=False):
    """Build the SPMD program. internal_io=True replaces the big I/O tensors
    with device-resident Internal DRAM (garbage contents) plus a dummy
    1-element output -- used only for wall-clock slope benchmarking where
    host<->device transfer must be excluded. repeat=N wraps the whole body
    in a For_i loop for slope timing. stages: subset for perf bisection
    (p1=projections, sc=scores+bias, sm=exp/norm, tr=transpose+attn-out,
    av=AV, p3=gating+o-proj)."""
    stages = set(stages)
    nc = bacc.Bacc("TRN2", target_bir_lowering=False, debug=False,
                   num_devices=NCORES)

    if internal_io:
        def din(name, shape, dt):
            return nc.dram_tensor(name, shape, dt).ap()
        def dout(name, shape, dt):
            return nc.dram_tensor(name, shape, dt).ap()
    else:
        def din(name, shape, dt):
            return nc.dram_tensor(name, shape, dt, kind="ExternalInput").ap()
        def dout(name, shape, dt):
            return nc.dram_tensor(name, shape, dt, kind="ExternalOutput").ap()

    xT = din("xT", (E, L), WDT)         # x[b].T
    xslT = din("xslT", (E, LS), WDT)    # x[b, lq_slice].T
    wqT = din("wqT", (E, E), WDT)       # scale * Wq^T, cols ordered (h, c)
    wkT = din("wkT", (E, E), WDT)
    wvT = din("wvT", (E, E), WDT)
    wgT = din("wgT", (E, E), WDT)
    woT = din("woT", (E, E), WDT)
    bgp = din("bgp", (P, EO), F32)      # bg[fo*128+p] at [p, fo]
    bob = din("bob", (P, E), F32)       # bo broadcast over partitions
    # declared f32r so it can be accumulated into the scores PSUM via an
    # identity matmul (bytes are plain fp32 from the host)
    biasT = din("biasT", (H, LS, L), F32R)  # bias[b,:,:,h] sliced+transposed
    attnT = dout("attnT", (L, H, LS), F32)
    yOut = dout("yOut", (LS, E), F32)
    if internal_io:
        marker = nc.dram_tensor("marker", (1, 1), F32,
                                kind="ExternalOutput").ap()

    xT_r = xT.rearrange("(eo p) l -> p eo l", p=P)
    xslT_r = xslT.rearrange("(eo p) l -> p eo l", p=P)
    wq_r = wqT.rearrange("(eo p) f -> p eo f", p=P)
    wk_r = wkT.rearrange("(eo p) f -> p eo f", p=P)
    wv_r = wvT.rearrange("(eo p) f -> p eo f", p=P)
    wg_r = wgT.rearrange("(eo p) f -> p eo f", p=P)
    wo_r = woT.rearrange("(eo p) f -> p eo f", p=P)
    biasT_r = biasT.rearrange("h (lqc p) lk -> h p lqc lk", p=P)
    attnT_r = attnT.rearrange("(lko p) h lq -> p lko h lq", p=P)
    yOut_r = yOut.rearrange("(lqc p) e -> p lqc e", p=P)

    with tile.TileContext(nc) as tc, ExitStack() as ctx:
        if repeat is not None:
            ctx.enter_context(tc.For_i(0, repeat))
        const = ctx.enter_context(tc.tile_pool(name="const", bufs=1))
        persist = ctx.enter_context(tc.tile_pool(name="persist", bufs=1))

        # f32r identity (0/1 exact): lhsT for f32r transposes
        identR = const.tile([P, P], F32R)
        bgp_sb = const.tile([P, EO], F32)
        nc.sync.dma_start(bgp_sb[:], bgp)

        # persistent intermediates
        kT_sb = persist.tile([P, EO, L], F32R)    # (f%128, fo, lk)
        v_sb = persist.tile([P, LKO, E], F32R)    # (lk%128, lko, f)
        qT_sb = persist.tile([P, EO, LS], F32R)   # (f%128, fo, lq)
        gT_sb = persist.tile([P, EO, LS], F32)
        yT_sb = persist.tile([P, EO, LS], F32)

        # tiny hoisted pool: head 0's bias prefetches during phase 1 (its
        # zone never overlaps phase-1 pools, so no release dependency)
        bias_pre = ctx.enter_context(tc.tile_pool(name="bias_pre", bufs=2))

        # ---------------- phase 1: projections ----------------
        with tc.tile_pool(name="xpool", bufs=1) as xpool, \
             tc.tile_pool(name="wpool", bufs=2) as wpool, \
             tc.tile_pool(name="ps1", bufs=4, space="PSUM") as ps1:
          ident = xpool.tile([P, P], F32)
          make_identity(nc, ident[:])
          nc.vector.tensor_copy(identR[:], ident[:])
          if "p1" in stages:
            xT_sb = xpool.tile([P, EO, L], WDT)
            wk_sb = wpool.tile([P, EO, E], WDT, tag="w")
            for eo in range(EO):
                nc.sync.dma_start(xT_sb[:, eo], xT_r[:, eo])
                nc.sync.dma_start(wk_sb[:, eo], wk_r[:, eo])
            xslT_sb = xpool.tile([P, EO, LS], WDT)
            nc.sync.dma_start(xslT_sb[:], xslT_r)

            # k^T: out (f-chunk, lk)
            for fo in range(EO):
                if ldw_amortize:
                    psts = [ps1.tile([P, 512], F32, tag="pst", name=f"kp{nh}") for nh in range(2)]
                    for eo in range(EO):
                        for nh in range(2):
                            nc.tensor.matmul(
                                psts[nh][:], wk_sb[:, eo, fo * P:(fo + 1) * P],
                                xT_sb[:, eo, nh * 512:(nh + 1) * 512],
                                start=(eo == 0), stop=(eo == EO - 1))
                    for nh in range(2):
                        if (fo + nh) % 2 == 0:
                            nc.scalar.copy(kT_sb[:, fo, nh * 512:(nh + 1) * 512], psts[nh][:])
                        else:
                            nc.vector.tensor_copy(kT_sb[:, fo, nh * 512:(nh + 1) * 512], psts[nh][:])
                else:
                    for nh in range(2):
                        pst = ps1.tile([P, 512], F32, tag="pst")
                        for eo in range(EO):
                            nc.tensor.matmul(
                                pst[:], wk_sb[:, eo, fo * P:(fo + 1) * P],
                                xT_sb[:, eo, nh * 512:(nh + 1) * 512],
                                start=(eo == 0), stop=(eo == EO - 1))
                        if (fo + nh) % 2 == 0:
                            nc.scalar.copy(kT_sb[:, fo, nh * 512:(nh + 1) * 512], pst[:])
                        else:
                            nc.vector.tensor_copy(kT_sb[:, fo, nh * 512:(nh + 1) * 512], pst[:])

            # v: out (lk-chunk, f)
            wv_sb = wpool.tile([P, EO, E], WDT, tag="w")
            for eo in range(EO):
                nc.sync.dma_start(wv_sb[:, eo], wv_r[:, eo])
            for lko in range(LKO):
                if ldw_amortize:
                    psts = [ps1.tile([P, 512], F32, tag="pst", name=f"vp{nh}") for nh in range(2)]
                    for eo in range(EO):
                        for nh in range(2):
                            nc.tensor.matmul(
                                psts[nh][:], xT_sb[:, eo, lko * P:(lko + 1) * P],
                                wv_sb[:, eo, nh * 512:(nh + 1) * 512],
                                start=(eo == 0), stop=(eo == EO - 1))
                    for nh in range(2):
                        if (lko + nh) % 2 == 0:
                            nc.scalar.copy(v_sb[:, lko, nh * 512:(nh + 1) * 512], psts[nh][:])
                        else:
                            nc.vector.tensor_copy(v_sb[:, lko, nh * 512:(nh + 1) * 512], psts[nh][:])
                else:
                    for nh in range(2):
                        pst = ps1.tile([P, 512], F32, tag="pst")
                        for eo in range(EO):
                            nc.tensor.matmul(
                                pst[:], xT_sb[:, eo, lko * P:(lko + 1) * P],
                                wv_sb[:, eo, nh * 512:(nh + 1) * 512],
                                start=(eo == 0), stop=(eo == EO - 1))
                        if (lko + nh) % 2 == 0:
                            nc.scalar.copy(v_sb[:, lko, nh * 512:(nh + 1) * 512], pst[:])
                        else:
                            nc.vector.tensor_copy(v_sb[:, lko, nh * 512:(nh + 1) * 512], pst[:])

            # q^T: out (f-chunk, lq_slice)
            wq_sb = wpool.tile([P, EO, E], WDT, tag="w")
            for eo in range(EO):
                nc.sync.dma_start(wq_sb[:, eo], wq_r[:, eo])
            for fo in range(EO):
                pst = ps1.tile([P, 512], F32, tag="pst")
                for eo in range(EO):
                    nc.tensor.matmul(
                        pst[:, :LS], wq_sb[:, eo, fo * P:(fo + 1) * P],
                        xslT_sb[:, eo, :],
                        start=(eo == 0), stop=(eo == EO - 1))
                if fo % 2 == 0:
                    nc.scalar.copy(qT_sb[:, fo, :], pst[:, :LS])
                else:
                    nc.vector.tensor_copy(qT_sb[:, fo, :], pst[:, :LS])

            # g^T with fused sigmoid(x@Wg^T + bg)
            wg_sb = wpool.tile([P, EO, E], WDT, tag="w")
            for eo in range(EO):
                nc.sync.dma_start(wg_sb[:, eo], wg_r[:, eo])
            for fo in range(EO):
                pst = ps1.tile([P, 512], F32, tag="pst")
                for eo in range(EO):
                    nc.tensor.matmul(
                        pst[:, :LS], wg_sb[:, eo, fo * P:(fo + 1) * P],
                        xslT_sb[:, eo, :],
                        start=(eo == 0), stop=(eo == EO - 1))
                nc.scalar.activation(gT_sb[:, fo, :], pst[:, :LS], AF.Sigmoid,
                                     bias=bgp_sb[:, fo:fo + 1])

        # o-proj weights loaded during phase 2 (pool sits in freed phase-1
        # space; was stalling the phase-3 tail ~20us when loaded at the end)
        ph3b = ctx.enter_context(tc.tile_pool(name="ph3b", bufs=1))
        wo_sb = ph3b.tile([P, EO, E], WDT, tag="w3")
        if "p3" in stages:
            nc.sync.dma_start(wo_sb[:], wo_r)

        # ---------------- phase 2: per-head attention ----------------
        _smb = 4 if deep_bufs else 3
        _pss = 2 if fused_bias else 4
        with tc.tile_pool(name="bias_p", bufs=6) as bias_p, \
             tc.tile_pool(name="sm", bufs=_smb) as sm, \
             tc.tile_pool(name="aT_p", bufs=3) as aT_p, \
             tc.tile_pool(name="zp", bufs=4) as zp, \
             tc.tile_pool(name="ps_s", bufs=_pss, space="PSUM") as ps_s, \
             tc.tile_pool(name="ps_t", bufs=2, space="PSUM") as ps_t, \
             tc.tile_pool(name="ps_y", bufs=2, space="PSUM") as ps_y:
            for h in range(H):
                hp, ho = (h % 2) * HW, h // 2
                bias_q = []
                for lqc in range(2):
                    _bp = bias_pre if h == 0 else bias_p
                    bq = _bp.tile([P, L], F32R, name=f"bq{lqc}", tag="bq")
                    bias_q.append(bq)
                    if "sc" in stages:
                        nc.sync.dma_start(bq[:], biasT_r[h][:, lqc])
                aT_sb = aT_p.tile([P, LKO, LS], F32R)
                for lqc in range(2):
                    s_sb = sm.tile([P, L], F32, tag="s")
                    if "sc" in stages and fused_bias:
                        pst = ps_s.tile([P, L], F32)
                        for lkh in range(2):
                            nc.tensor.matmul(
                                pst[:, lkh * 512:(lkh + 1) * 512],
                                qT_sb[hp:hp + HW, ho, lqc * P:(lqc + 1) * P],
                                kT_sb[hp:hp + HW, ho, lkh * 512:(lkh + 1) * 512],
                                start=True, stop=True)
                        beng = nc.gpsimd if (gp_bias and h % 2 == 1) else nc.vector
                        beng.tensor_tensor(
                            s_sb[:], pst[:],
                            bias_q[lqc][:].bitcast(F32), ALU.add)
                    elif "sc" in stages:
                        for lkh in range(2):
                            pst = ps_s.tile([P, 512], F32)
                            nc.tensor.matmul(
                                pst[:],
                                qT_sb[hp:hp + HW, ho, lqc * P:(lqc + 1) * P],
                                kT_sb[hp:hp + HW, ho, lkh * 512:(lkh + 1) * 512],
                                start=True, stop=True)
                            beng = nc.gpsimd if (gp_bias and h % 2 == 1) else nc.vector
                            beng.tensor_tensor(
                                s_sb[:, lkh * 512:(lkh + 1) * 512], pst[:],
                                bias_q[lqc][:, lkh * 512:(lkh + 1) * 512].bitcast(F32),
                                ALU.add)
                    e_sb = sm.tile([P, L], F32R, tag="e")
                    if "sm" in stages:
                        zc = zp.tile([P, 1], F32, tag="z")
                        rz = zp.tile([P, 1], F32, tag="rz")
                        nc.scalar.activation(e_sb[:], s_sb[:], AF.Exp,
                                             accum_out=zc[:])
                        nc.vector.reciprocal(rz[:], zc[:])
                        if act_norm and h % 2 == 1:
                            nc.scalar.mul(e_sb[:], e_sb[:], rz[:, 0:1])
                        else:
                            nc.vector.tensor_scalar_mul(e_sb[:], e_sb[:], rz[:])
                    if "tr" in stages:
                        # transpose in pairs -> one (128, 256) psum tile,
                        # one copyback per pair, alternating DVE/ACT
                        for lkp in range(LKO // 2):
                            tp = ps_t.tile([P, 2 * P], F32R)
                            for j in range(2):
                                nc.tensor.transpose(
                                    tp[:, j * P:(j + 1) * P],
                                    e_sb[:, (2 * lkp + j) * P:(2 * lkp + j + 1) * P],
                                    identR[:])
                            dst = aT_sb[:, 2 * lkp:2 * lkp + 2,
                                        lqc * P:(lqc + 1) * P]
                            src = tp[:].rearrange("p (k l) -> p k l", k=2)
                            if lkp % 2 == 0:
                                nc.vector.tensor_copy(dst, src)
                            else:
                                nc.scalar.copy(dst, src)
                # AV: y^T_h (c=64, lq)
                if "av" in stages:
                    yp = ps_y.tile([HW, LS], F32)
                    for lko in range(LKO):
                        nc.tensor.matmul(
                            yp[:], v_sb[:, lko, h * HW:(h + 1) * HW],
                            aT_sb[:, lko, :],
                            start=(lko == 0), stop=(lko == LKO - 1))
                    nc.scalar.copy(yT_sb[hp:hp + HW, ho, :], yp[:])
                if "tr" in stages:
                    nc.gpsimd.dma_start(attnT_r[:, :, h, :], aT_sb[:].bitcast(F32))

        # ---------------- phase 3: gating + o-proj ----------------
        with tc.tile_pool(name="ph3c", bufs=2) as ph3c, \
             tc.tile_pool(name="ps3", bufs=4, space="PSUM") as ps3:
          if "p3" in stages:
            bob_sb = ph3c.tile([P, E], F32, tag="bob")
            nc.sync.dma_start(bob_sb[:], bob)
            ygT = ph3c.tile([P, EO, LS], WDT, tag="yg")
            nc.vector.tensor_tensor(ygT[:], yT_sb[:], gT_sb[:], ALU.mult)
            for lqc in range(2):
                for nh in range(2):
                    pst = ps3.tile([P, 512], F32)
                    for fo in range(EO):
                        nc.tensor.matmul(
                            pst[:], ygT[:, fo, lqc * P:(lqc + 1) * P],
                            wo_sb[:, fo, nh * 512:(nh + 1) * 512],
                            start=(fo == 0), stop=(fo == EO - 1))
                    o_sb = ph3c.tile([P, 512], F32, tag="o")
                    nc.vector.tensor_tensor(
                        o_sb[:], pst[:], bob_sb[:, nh * 512:(nh + 1) * 512],
                        ALU.add)
                    nc.sync.dma_start(yOut_r[:, lqc, nh * 512:(nh + 1) * 512],
                                      o_sb[:])

        if internal_io:
            one = const.tile([1, 1], F32)
            nc.vector.memset(one[:], 1.0)
            nc.sync.dma_start(marker[:], one[:])

    nc.compile()
    return nc


def _get_nc():
    global _NC_CACHE
    if _NC_CACHE is None:
        _NC_CACHE = _build()
    return _NC_CACHE


def kernel(**inputs):
    x = np.ascontiguousarray(np.asarray(inputs["x"]), dtype=np.float32)
    mask = np.asarray(inputs["mask"])
    bias = np.asarray(inputs["bias"], dtype=np.float32)
    Wqkv = np.asarray(inputs["Wqkv"], dtype=np.float32)
    Wo = np.asarray(inputs["Wo"], dtype=np.float32)
    bo = np.asarray(inputs["bo"], dtype=np.float32)
    Wg = np.asarray(inputs["Wg"], dtype=np.float32)
    bg = np.asarray(inputs["bg"], dtype=np.float32)

    import ml_dtypes
    wnp = ml_dtypes.bfloat16 if WDT == BF16 else np.float32
    scale = HW ** -0.5
    Wr = Wqkv.reshape(H, 3, HW, E)
    wqT = np.ascontiguousarray((Wr[:, 0].reshape(H * HW, E) * scale).T.astype(wnp))
    wkT = np.ascontiguousarray(Wr[:, 1].reshape(H * HW, E).T.astype(wnp))
    wvT = np.ascontiguousarray(Wr[:, 2].reshape(H * HW, E).T.astype(wnp))
    wgT = np.ascontiguousarray(Wg.T.astype(wnp))
    woT = np.ascontiguousarray(Wo.T.astype(wnp))
    bgp = np.ascontiguousarray(bg.reshape(EO, P).T)
    bob = np.ascontiguousarray(np.broadcast_to(bo, (P, E)))

    if not mask.all():
        # masked keys -> -1e30 on their score columns (exp underflows to 0,
        # matching the reference's -inf semantics)
        bias = bias + np.where(mask[:, None, :, None], 0.0, -1e30).astype(
            np.float32)

    in_maps = []
    for c in range(NCORES):
        b, s = divmod(c, SLICES)
        bT = np.ascontiguousarray(
            bias[b].transpose(2, 0, 1)[:, s * LS:(s + 1) * LS, :])
        in_maps.append({
            "xT": np.ascontiguousarray(x[b].T.astype(wnp)),
            "xslT": np.ascontiguousarray(x[b, s * LS:(s + 1) * LS].T.astype(wnp)),
            "wqT": wqT, "wkT": wkT, "wvT": wvT, "wgT": wgT, "woT": woT,
            "bgp": bgp, "bob": bob,
            "biasT": bT,
        })

    nc = _get_nc()
    res = run_bass_kernel_spmd(nc, in_maps, core_ids=list(range(NCORES)))

    y = np.empty((B, L, E), dtype=np.float32)
    attn = np.empty((B, L, H, L), dtype=np.float32)
    for c in range(NCORES):
        b, s = divmod(c, SLICES)
        y[b, s * LS:(s + 1) * LS] = res.results[c]["yOut"]
        attn[b, :, :, s * LS:(s + 1) * LS] = res.results[c]["attnT"]
    return y, attn
